# revision 2
# baseline (speedup 1.0000x reference)
"""Trainium2 Bass kernel for nn_AttentionBlock (MLA-style attention + SwiGLU FFN).

v2 restructure (sim-driven):
  - nb-streamed projections (512-key/query chunks, double-buffered DMA) so
    activations prefetch across phases and batches; weight tiles loaded once.
  - attention uses 1024-query score blocks (one exp per [128,1024] tile),
    softmax denominators accumulated on DVE (f16) + single ones-matmul,
    denominator broadcast via a rank-1 matmul instead of DRAM round-trips.
  - W_O partial-output matmuls interleaved per 1024-query block with the
    attention compute; psum evacuation on DVE; deep DMA write buffering.
  - FFN bridge (gate/up on batch-0 half tokens) widened to NBRQ quads to
    hide the second reduce-scatter; down-projection double-buffers psum
    groups of 4 banks to overlap evacuation with matmuls.
All matmuls bf16/f16 with fp32 PSUM accumulation.
"""
import sys
sys.path.insert(0, '/opt/trn_rl_repo')

import math
import numpy as np
import ml_dtypes

from concourse import bass, bacc, mybir, tile
from concourse.bass_utils import run_bass_kernel_spmd

# ---- inlined wait pruner (kernel.py must be self-contained) ----
import bisect


def _is_dma(inst):
    return type(inst).__name__ in (
        "InstDMACopy", "InstDmaTranspose", "InstDmaTransposeAnt",
        "InstTensorCopyDma", "InstTensorReduceDma")


def prune_redundant_waits(nc, verbose=False):
    insts = []
    for f in nc.m.functions:
        for blk in f.blocks:
            insts.extend(blk.instructions)

    poisoned = set()
    running = {}
    producers = {}   # sem -> ([values], [idx])
    VC = [None] * len(insts)
    chain_vc = {}    # engine -> completion vc of last instruction
    chain_prev = [None] * len(insts)   # vc inherited from chain (pre-wait)

    def producer_at_least(sem, v):
        if sem in poisoned or sem not in producers:
            return None
        vals, idxs = producers[sem]
        i = bisect.bisect_left(vals, v)
        if i == len(vals):
            return None
        return vals[i], idxs[i]

    def merge(dst, src):
        for s, v in src.items():
            if dst.get(s, -1) < v:
                dst[s] = v

    for idx, inst in enumerate(insts):
        si = inst.sync_info
        is_dma = _is_dma(inst)
        ekey = getattr(inst, "engine", None)
        if is_dma:
            vc = {}
        else:
            vc = dict(chain_vc.get(ekey, {}))
        chain_prev[idx] = dict(vc)
        if si is not None:
            for w in si.on_wait:
                if w.wait_mode != "sem-ge-imm" or w.id in poisoned:
                    continue
                p = producer_at_least(w.id, w.wait_value)
                if p is not None:
                    merge(vc, VC[p[1]])
                    if vc.get(w.id, -1) < p[0]:
                        vc[w.id] = p[0]
                else:
                    if vc.get(w.id, -1) < w.wait_value:
                        vc[w.id] = w.wait_value
            for u in si.on_update:
                if u.update_mode in ("sem-inc", "sem-add-imm"):
                    nv = running.get(u.id, 0) + u.update_value
                    running[u.id] = nv
                    producers.setdefault(u.id, ([], []))
                    producers[u.id][0].append(nv)
                    producers[u.id][1].append(idx)
                    if vc.get(u.id, -1) < nv:
                        vc[u.id] = nv
                else:
                    poisoned.add(u.id)
        VC[idx] = vc
        if not is_dma:
            chain_vc[ekey] = vc

    # pass 2: prune
    n_pruned = 0
    for idx, inst in enumerate(insts):
        si = inst.sync_info
        if si is None or len(si.on_wait) < 2:
            continue
        waits = list(si.on_wait)
        kept = list(waits)
        changed = True
        while changed and len(kept) > 1:
            changed = False
            for w in kept:
                if w.wait_mode != "sem-ge-imm" or w.id in poisoned:
                    continue
                cover = dict(chain_prev[idx])
                ok_others = True
                for o in kept:
                    if o is w:
                        continue
                    if o.wait_mode != "sem-ge-imm" or o.id in poisoned:
                        continue
                    p = producer_at_least(o.id, o.wait_value)
                    if p is not None:
                        merge(cover, VC[p[1]])
                if cover.get(w.id, -1) >= w.wait_value:
                    kept.remove(w)
                    n_pruned += 1
                    changed = True
                    break
        if len(kept) != len(waits):
            import concourse.mybir as mybir
            inst.sync_info = mybir.SyncInfo(on_wait=kept, on_update=list(si.on_update))
    if verbose:
        print(f"wait_prune: removed {n_pruned} waits")
    return n_pruned

# ---- end wait pruner ----


BF = mybir.dt.bfloat16
F16 = mybir.dt.float16
F32 = mybir.dt.float32
AF = mybir.ActivationFunctionType
AO = mybir.AluOpType

D = 2048
N_H = 16
D_H = 128
D_R = 64
FFN = 8192
THETA = 1000000.0
EPS = 1e-6
SCALE = 1.0 / math.sqrt(D_H + D_R)
NCORES = 8
P = 128
TOK = 512          # tokens per core in the FFN phase
NBRQ = 8           # gate/up quads bridged onto the b0-token half


# --------------------------------------------------------------------------
# Fused single-launch kernel
# --------------------------------------------------------------------------
def build_fused(B, M, N, Dm, HPC, DH=D_H, DR=D_R):
    DC = Dm // P          # 16 contraction chunks over D
    NT = N // P           # 16 key chunks
    NB = 512              # streamed key/query chunk
    NNB = N // NB         # 4
    MBB = 1024            # attention query block
    NMB = M // MBB        # 2
    RD = HPC * DR         # 128
    HD = HPC * DH         # 256
    FB = FFN // P         # 64 ffn blocks
    TOKB = TOK // 2       # 256 tokens per (core, batch)
    ln_scale_bias = float(math.log(SCALE))

    nc = bacc.Bacc(num_devices=NCORES)
    qT = nc.dram_tensor("qT", [B, Dm, M], BF, kind="ExternalInput")
    kvT = nc.dram_tensor("kvT", [B, Dm, N], BF, kind="ExternalInput")
    wq = nc.dram_tensor("wq", [Dm, HD], BF, kind="ExternalInput")
    wqr = nc.dram_tensor("wqr", [Dm, RD], BF, kind="ExternalInput")
    wk = nc.dram_tensor("wk", [Dm, HD], BF, kind="ExternalInput")
    wkr = nc.dram_tensor("wkr", [Dm, RD], BF, kind="ExternalInput")
    wv = nc.dram_tensor("wv", [Dm, HD], BF, kind="ExternalInput")
    wo = nc.dram_tensor("wo", [HD, Dm], F16, kind="ExternalInput")
    cos2T = nc.dram_tensor("cos2T", [RD, M], F16, kind="ExternalInput")
    sin2T = nc.dram_tensor("sin2T", [RD, M], F16, kind="ExternalInput")
    rot2T = nc.dram_tensor("rot2T", [RD, RD], BF, kind="ExternalInput")
    qres = nc.dram_tensor("qres", [Dm, TOK], F16, kind="ExternalInput")
    wg = nc.dram_tensor("wg", [Dm, FFN], BF, kind="ExternalInput")
    wu = nc.dram_tensor("wu", [Dm, FFN], BF, kind="ExternalInput")
    wd = nc.dram_tensor("wd", [FFN, Dm], BF, kind="ExternalInput")
    yT = nc.dram_tensor("yT", [Dm, TOK], F32, kind="ExternalOutput")

    with tile.TileContext(nc) as tc:
      with tc.tile_pool(name="const", bufs=1) as cp, \
           tc.tile_pool(name="dram", bufs=1, space="DRAM") as dramp:
        ones_bf = cp.tile([P, 1], BF, tag="ones")
        nc.vector.memset(ones_bf[:], 1.0)
        ones_f16c = cp.tile([P, 1], F16, tag="ones16")
        nc.vector.memset(ones_f16c[:], 1.0)
        ones_row = cp.tile([1, P], F16, tag="onesrow")
        nc.vector.memset(ones_row[:], 1.0)
        eps_t = cp.tile([P, 1], F32, tag="eps")
        nc.vector.memset(eps_t[:], EPS)
        lnsc_t = cp.tile([P, 1], F32, tag="lnsc")
        nc.vector.memset(lnsc_t[:], ln_scale_bias)

        # dest-major partial x^T buffers (one per batch) and their
        # reduce-scattered results
        po_pks = [dramp.tile([NCORES * Dm, TOKB], F16, tag=f"po_pk{b}",
                             name=f"po_pk{b}") for b in range(B)]
        xpTs = [dramp.tile([Dm, TOKB], F16, tag=f"xpT{b}", name=f"xpT{b}")
                for b in range(B)]

        attn_cm = tc.tile_pool(name="attnc", bufs=1)
        acp = attn_cm.__enter__()
        cosT_sb = acp.tile([RD, M], F16, tag="cos")
        sinT_sb = acp.tile([RD, M], F16, tag="sin")
        rot_sb = acp.tile([RD, RD], BF, tag="proj")
        nc.sync.dma_start(out=cosT_sb[:], in_=cos2T[:])
        nc.sync.dma_start(out=sinT_sb[:], in_=sin2T[:])
        nc.sync.dma_start(out=rot_sb[:], in_=rot2T[:])

        # -- deferred residual+rms state --
        resid = {}

        def emit_resid_half(half):
            lo, hi = half * TOKB, (half + 1) * TOKB
            xTf, xnT, nbc = resid['xTf'], resid['xnT'], resid['nbc']
            rpp, rsp, dnr = resid['rpp'], resid['rsp'], resid['dnr']
            rtag = resid['rtag']
            ssq = rpp.tile([1, TOKB], F32, tag=rtag, name=f"ssq{half}")
            xp_a = rsp.tile([P, DC * TOKB], F16, tag="xph", bufs=1, name="xp_a")
            nc.sync.dma_start(
                out=xp_a[:].rearrange("p (c f) -> p c f", c=DC),
                in_=xpTs[half][:].rearrange("(c p) f -> p c f", p=P))
            qr_a = rsp.tile([P, DC * TOKB], F16, tag="qrh", bufs=1, name="qr_a")
            nc.sync.dma_start(
                out=qr_a[:].rearrange("p (c f) -> p c f", c=DC),
                in_=qres[:, lo:hi].rearrange("(c p) f -> p c f", p=P))
            for db in range(DC):
                dsl = slice(db * TOKB, (db + 1) * TOKB)
                nc.vector.tensor_add(xTf[db][:, lo:hi], xp_a[:, dsl],
                                     qr_a[:, dsl])
                sq = rsp.tile([P, TOKB], BF, tag="sqh", name="sqh")
                nc.vector.tensor_mul(sq[:], xTf[db][:, lo:hi],
                                     xTf[db][:, lo:hi])
                nc.tensor.matmul(ssq[:], ones_bf[:], sq[:],
                                 start=(db == 0), stop=(db == DC - 1))
            nrow = rsp.tile([1, TOKB], F32, tag="nrh", name="nrh")
            nc.scalar.activation(nrow[:], ssq[:], AF.Ln,
                                 scale=1.0 / Dm, bias=eps_t[0:1, :])
            nrow2 = rsp.tile([1, TOKB], F16, tag="nr2h", name="nr2h")
            nc.scalar.activation(nrow2[:], nrow[:], AF.Exp, scale=-0.5)
            nc.sync.dma_start(out=dnr[0:1, lo:hi], in_=nrow2[:])
            nc.sync.dma_start(out=nbc[:, lo:hi],
                              in_=dnr[0:1, lo:hi].to_broadcast((P, TOKB)))
            for db in range(DC):
                nc.vector.tensor_mul(xnT[db][:, lo:hi], xTf[db][:, lo:hi],
                                     nbc[:, lo:hi])

        xf_cm = tc.tile_pool(name="xf", bufs=1)
        xf = xf_cm.__enter__()
        rs_cm = tc.tile_pool(name="rss", bufs=3)
        resid['rsp'] = rs_cm.__enter__()
        resid['xTf'] = [xf.tile([P, TOK], F16, tag=f"xT{i}",
                                name=f"xT{i}") for i in range(DC)]
        resid['xnT'] = [xf.tile([P, TOK], BF, tag=f"xn{i}",
                                name=f"xn{i}") for i in range(DC)]
        resid['nbc'] = xf.tile([P, TOK], F16, tag="nbc", name="nbc")
        resid['dnr'] = dramp.tile([1, TOK], F16, tag="dnr", name="dnr")

        # -------- persistent weight + streamed activation pools --------
        wp_cm = tc.tile_pool(name="projw", bufs=1)
        wpp = wp_cm.__enter__()

        def load_stacked(dst, src, W):
            # dst[p, dc*W + j] = src[dc*P + p, j] in one DMA
            nc.sync.dma_start(
                out=dst[:].rearrange("p (c f) -> p c f", c=DC),
                in_=src[:].rearrange("(c p) f -> p c f", p=P))

        wkt_a = wpp.tile([P, DC * HD], BF, tag="wk", name="wkt_a")
        wkrt_a = wpp.tile([P, DC * RD], BF, tag="wkr", name="wkrt_a")
        wvt_a = wpp.tile([P, DC * HD], BF, tag="wv", name="wvt_a")
        wqt_a = wpp.tile([P, DC * HD], BF, tag="wq", name="wqt_a")
        wqrt_a = wpp.tile([P, DC * RD], BF, tag="wqr", name="wqrt_a")
        load_stacked(wkt_a, wk, HD)
        load_stacked(wkrt_a, wkr, RD)
        load_stacked(wvt_a, wv, HD)
        load_stacked(wqt_a, wq, HD)
        load_stacked(wqrt_a, wqr, RD)
        wkt = [wkt_a[:, i * HD:(i + 1) * HD] for i in range(DC)]
        wkrt = [wkrt_a[:, i * RD:(i + 1) * RD] for i in range(DC)]
        wvt = [wvt_a[:, i * HD:(i + 1) * HD] for i in range(DC)]
        wqt = [wqt_a[:, i * HD:(i + 1) * HD] for i in range(DC)]
        wqrt = [wqrt_a[:, i * RD:(i + 1) * RD] for i in range(DC)]

        kvs_cm = tc.tile_pool(name="kvstream", bufs=2)
        kvsp = kvs_cm.__enter__()
        qsp = kvsp  # q chunks share the kv stream slots (alternating units)

        for b in range(B):
          with tc.tile_pool(name=f"kq{b}", bufs=1) as kq:
            kt = [kq.tile([P, N], BF, tag=f"kt{h}", name=f"kt{h}") for h in range(HPC)]
            krt = kq.tile([RD, N], BF, tag="krt")
            vt = [kq.tile([P, HD], F16, tag=f"vt{i}", name=f"vt{i}") for i in range(NT)]
            qt = [kq.tile([P, M], BF, tag=f"qt{h}", name=f"qt{h}") for h in range(HPC)]
            qrt = kq.tile([RD, M], BF, tag="qrt")

            with tc.tile_pool(name=f"pp{b}", bufs=2, space="PSUM") as prp, \
                 tc.tile_pool(name=f"sc{b}", bufs=2) as sp:
                drow = dramp.tile([1, N], F32, tag=f"dkv{b}", name=f"dkv{b}")
                # ================= KV side (nb-streamed) =================
                for nb in range(NNB):
                    nbs = slice(nb * NB, (nb + 1) * NB)
                    kvc_a = kvsp.tile([P, DC * NB], BF, tag="kv", name="kvc_a")
                    nc.sync.dma_start(
                        out=kvc_a[:].rearrange("p (c f) -> p c f", c=DC),
                        in_=kvT[b, :, nbs].rearrange("(c p) f -> p c f", p=P))
                    kvc = [kvc_a[:, i * NB:(i + 1) * NB] for i in range(DC)]
                    # rms stats
                    ssq = prp.tile([1, NB], F32, tag="ssq", name="ssq")
                    for dc in range(DC):
                        sq = sp.tile([P, NB], BF, tag="sq", name="sq")
                        nc.vector.tensor_mul(sq[:], kvc[dc][:], kvc[dc][:])
                        nc.tensor.matmul(ssq[:], ones_bf[:], sq[:],
                                         start=(dc == 0), stop=(dc == DC - 1))
                    ln_t = sp.tile([1, NB], F32, tag="lnt", name="lnt")
                    nc.scalar.activation(ln_t[:], ssq[:], AF.Ln,
                                         scale=1.0 / Dm, bias=eps_t[0:1, :])
                    nc.sync.dma_start(out=drow[0:1, nbs], in_=ln_t[:])
                    nkv_row = sp.tile([1, NB], F16, tag="nkvrow", name="nkvrow")
                    nc.scalar.activation(nkv_row[:], ln_t[:], AF.Exp,
                                         scale=-0.5, bias=lnsc_t[0:1, :])
                    bcps = prp.tile([P, NB], F32, tag="bc", name="bcps")
                    nc.tensor.matmul(bcps[:], ones_row[:], nkv_row[:],
                                     start=True, stop=True)
                    nkv_bc = sp.tile([P, NB], F16, tag="nkvbc", name="nkvbc")
                    nc.vector.tensor_copy(nkv_bc[:], bcps[:])
                    # V-evac per-key norm column (no SCALE)
                    lncol = sp.tile([P, NB // P], F32, tag="lncol", name="lncol")
                    nc.sync.dma_start(
                        out=lncol[:],
                        in_=drow[0:1, nbs].rearrange("a (t p) -> (a p) t", p=P))
                    nkvV = sp.tile([P, NB // P], F32, tag="nkvv", name="nkvv")
                    nc.scalar.activation(nkvV[:], lncol[:], AF.Exp, scale=-0.5)

                    # K_C projection
                    for h in range(HPC):
                        ps = prp.tile([P, NB], F32, tag="proj", name="ps")
                        for dc in range(DC):
                            nc.tensor.matmul(ps[:], wkt[dc][:, h * DH:(h + 1) * DH],
                                             kvc[dc][:],
                                             start=(dc == 0), stop=(dc == DC - 1))
                        nc.vector.tensor_mul(kt[h][:, nbs], ps[:], nkv_bc[:])
                    # K_R projection
                    ps = prp.tile([RD, NB], F32, tag="proj", name="ps")
                    for dc in range(DC):
                        nc.tensor.matmul(ps[:], wkrt[dc][:], kvc[dc][:],
                                         start=(dc == 0), stop=(dc == DC - 1))
                    krr = sp.tile([RD, NB], BF, tag="krraw", name="krr")
                    nc.vector.tensor_mul(krr[:], ps[:], nkv_bc[:RD, :])
                    # rope K
                    rps = prp.tile([RD, NB], F32, tag="proj", name="rps")
                    nc.tensor.matmul(rps[:], rot_sb[:], krr[:],
                                     start=True, stop=True)
                    c_t = sp.tile([RD, NB], BF, tag="ropec", name="c_t")
                    nc.vector.tensor_mul(c_t[:], krr[:], cosT_sb[:, nbs])
                    s_t = sp.tile([RD, NB], BF, tag="ropes", name="s_t")
                    nc.vector.tensor_mul(s_t[:], rps[:], sinT_sb[:, nbs])
                    nc.vector.tensor_add(krt[:, nbs], c_t[:], s_t[:])
                    # V projection (keys stationary)
                    for j in range(NB // P):
                        nt = nb * (NB // P) + j
                        ps2 = prp.tile([P, HD], F32, tag="projv", name="ps2")
                        for dc in range(DC):
                            nc.tensor.matmul(ps2[:],
                                             kvc[dc][:, j * P:(j + 1) * P],
                                             wvt[dc][:],
                                             start=(dc == 0), stop=(dc == DC - 1))
                        nc.vector.tensor_scalar_mul(vt[nt][:], ps2[:],
                                                    nkvV[:, j:j + 1])

                # ================= Q side (nb-streamed) =================
                for nb in range(NNB):
                    nbs = slice(nb * NB, (nb + 1) * NB)
                    qc_a = qsp.tile([P, DC * NB], BF, tag="kv", name="qc_a")
                    nc.sync.dma_start(
                        out=qc_a[:].rearrange("p (c f) -> p c f", c=DC),
                        in_=qT[b, :, nbs].rearrange("(c p) f -> p c f", p=P))
                    qc = [qc_a[:, i * NB:(i + 1) * NB] for i in range(DC)]
                    ssq = prp.tile([1, NB], F32, tag="ssq", name="ssq")
                    for dc in range(DC):
                        sq = sp.tile([P, NB], BF, tag="sq", name="sq")
                        nc.vector.tensor_mul(sq[:], qc[dc][:], qc[dc][:])
                        nc.tensor.matmul(ssq[:], ones_bf[:], sq[:],
                                         start=(dc == 0), stop=(dc == DC - 1))
                    ln_t = sp.tile([1, NB], F32, tag="lnt", name="lnt")
                    nc.scalar.activation(ln_t[:], ssq[:], AF.Ln,
                                         scale=1.0 / Dm, bias=eps_t[0:1, :])
                    nq_row = sp.tile([1, NB], F16, tag="nkvrow", name="nq_row")
                    nc.scalar.activation(nq_row[:], ln_t[:], AF.Exp, scale=-0.5)
                    bcps = prp.tile([P, NB], F32, tag="bc", name="bcps")
                    nc.tensor.matmul(bcps[:], ones_row[:], nq_row[:],
                                     start=True, stop=True)
                    nq_bc = sp.tile([P, NB], F16, tag="nkvbc", name="nq_bc")
                    nc.vector.tensor_copy(nq_bc[:], bcps[:])

                    for h in range(HPC):
                        ps = prp.tile([P, NB], F32, tag="proj", name="ps")
                        for dc in range(DC):
                            nc.tensor.matmul(ps[:], wqt[dc][:, h * DH:(h + 1) * DH],
                                             qc[dc][:],
                                             start=(dc == 0), stop=(dc == DC - 1))
                        nc.vector.tensor_mul(qt[h][:, nbs], ps[:], nq_bc[:])
                    ps = prp.tile([RD, NB], F32, tag="proj", name="ps")
                    for dc in range(DC):
                        nc.tensor.matmul(ps[:], wqrt[dc][:], qc[dc][:],
                                         start=(dc == 0), stop=(dc == DC - 1))
                    qrr = sp.tile([RD, NB], BF, tag="krraw", name="qrr")
                    nc.vector.tensor_mul(qrr[:], ps[:], nq_bc[:RD, :])
                    rps = prp.tile([RD, NB], F32, tag="proj", name="rps")
                    nc.tensor.matmul(rps[:], rot_sb[:], qrr[:],
                                     start=True, stop=True)
                    c_t = sp.tile([RD, NB], BF, tag="ropec", name="c_t")
                    nc.vector.tensor_mul(c_t[:], qrr[:], cosT_sb[:, nbs])
                    s_t = sp.tile([RD, NB], BF, tag="ropes", name="s_t")
                    nc.vector.tensor_mul(s_t[:], rps[:], sinT_sb[:, nbs])
                    nc.vector.tensor_add(qrt[:, nbs], c_t[:], s_t[:])

            # ================= attention + interleaved W_O =================
            with tc.tile_pool(name=f"at{b}", bufs=1) as ap, \
                 tc.tile_pool(name=f"ap{b}", bufs=2, space="PSUM") as pp, \
                 tc.tile_pool(name=f"ae{b}", bufs=4) as ep, \
                 tc.tile_pool(name=f"ao{b}", bufs=2) as osb:
                wo_sb = [ap.tile([P, Dm], F16, tag=f"wo{h}", name=f"wo{h}")
                         for h in range(HPC)]
                for h in range(HPC):
                    nc.sync.dma_start(out=wo_sb[h][:],
                                      in_=wo[h * DH:(h + 1) * DH, :])

                for mb in range(NMB):
                    mbs = slice(mb * MBB, (mb + 1) * MBB)
                    ut = [ap.tile([P, MBB], F16, tag=f"ut{h}", bufs=1,
                                  name=f"ut{h}") for h in range(HPC)]
                    u_pss, accs = [], []
                    for h in range(HPC):
                        u_ps = pp.tile([P, MBB], F32, tag="u", name="u_ps")
                        acc = ap.tile([P, MBB], F16, tag="acc", bufs=2,
                                      name="acc")
                        u_pss.append(u_ps)
                        accs.append(acc)
                        for nt in range(NT):
                            s_ps = pp.tile([P, MBB], F32, tag="s", name="s_ps")
                            for hf in range(2):
                                sl = slice(hf * 512, (hf + 1) * 512)
                                qs_ = slice(mb * MBB + hf * 512,
                                            mb * MBB + (hf + 1) * 512)
                                nc.tensor.matmul(
                                    s_ps[:, sl], kt[h][:, nt * P:(nt + 1) * P],
                                    qt[h][:, qs_], start=True, stop=False)
                                nc.tensor.matmul(
                                    s_ps[:, sl],
                                    krt[h * DR:(h + 1) * DR, nt * P:(nt + 1) * P],
                                    qrt[h * DR:(h + 1) * DR, qs_],
                                    start=False, stop=True)
                            et = ep.tile([P, MBB], F16, tag="et", name="et")
                            nc.scalar.activation(et[:], s_ps[:], AF.Exp)
                            for hf in range(2):
                                sl = slice(hf * 512, (hf + 1) * 512)
                                nc.tensor.matmul(u_ps[:, sl],
                                                 vt[nt][:, h * DH:(h + 1) * DH],
                                                 et[:, sl], start=(nt == 0),
                                                 stop=(nt == NT - 1))
                            with nc.allow_low_precision(
                                    reason="f16 softmax denom accumulate"):
                                if nt == 0:
                                    nc.vector.tensor_copy(acc[:], et[:])
                                else:
                                    nc.vector.tensor_add(acc[:], acc[:], et[:])
                    for h in range(HPC):
                        u_ps, acc = u_pss[h], accs[h]
                        srow_ps = pp.tile([P, MBB], F32, tag="s", name="srow_ps")
                        for hf in range(2):
                            sl = slice(hf * 512, (hf + 1) * 512)
                            nc.tensor.matmul(srow_ps[0:1, sl], ones_f16c[:],
                                             acc[:, sl], start=True, stop=True)
                        srow_sb = ap.tile([1, MBB], F16, tag="srow", bufs=2,
                                          name="srow_sb")
                        nc.scalar.copy(srow_sb[:], srow_ps[0:1, :])
                        bc_ps = pp.tile([P, MBB], F32, tag="s", name="bc_ps")
                        for hf in range(2):
                            sl = slice(hf * 512, (hf + 1) * 512)
                            nc.tensor.matmul(bc_ps[:, sl], ones_row[:],
                                             srow_sb[0:1, sl],
                                             start=True, stop=True)
                        rin = ap.tile([P, MBB], F16, tag="rin", bufs=1,
                                      name="rin")
                        with nc.allow_low_precision(
                                reason="f16 softmax reciprocal"):
                            nc.vector.reciprocal(rin[:], bc_ps[:])
                        nc.vector.tensor_mul(ut[h][:], u_ps[:], rin[:])

                    # W_O partials for this query block
                    for j in range(MBB // TOKB):
                        dest = mb * (MBB // TOKB) + j
                        tsl = slice(j * TOKB, (j + 1) * TOKB)
                        for dh in range(2):
                            o_all = osb.tile([P, DC // 2 * TOKB], F16,
                                             tag="osb", name="o_all")
                            for g in range(2):
                                wp4 = pp.tile([P, MBB], F32, tag="s",
                                              name="wp4")
                                for di in range(4):
                                    db = dh * (DC // 2) + g * 4 + di
                                    dsl = slice(di * TOKB, (di + 1) * TOKB)
                                    for h in range(HPC):
                                        nc.tensor.matmul(
                                            wp4[:, dsl],
                                            wo_sb[h][:, db * P:(db + 1) * P],
                                            ut[h][:, tsl],
                                            start=(h == 0),
                                            stop=(h == HPC - 1))
                                osl = slice(g * MBB, (g + 1) * MBB)
                                if g == 0:
                                    nc.vector.tensor_copy(o_all[:, osl],
                                                          wp4[:])
                                else:
                                    nc.scalar.copy(o_all[:, osl], wp4[:])
                            r0 = dest * Dm + dh * (DC // 2) * P
                            nc.gpsimd.dma_start(
                                out=po_pks[b][r0:r0 + DC // 2 * P, :]
                                    .rearrange("(c p) f -> p c f", p=P),
                                in_=o_all[:].rearrange("p (c f) -> p c f",
                                                       c=DC // 2))

                if b == 1:
                    # batch-0 residual+rms while RS(b1) is about to fly;
                    # RS(b0) is ~2 full phases old by now.
                    resid['rpp'] = pp
                    resid['rtag'] = "s"
                    emit_resid_half(0)

          nc.gpsimd.collective_compute(
              "ReduceScatter", AO.add,
              replica_groups=[list(range(NCORES))],
              ins=[po_pks[b].opt()], outs=[xpTs[b].opt()])

        kvs_cm.__exit__(None, None, None)
        wp_cm.__exit__(None, None, None)

        # ============ bridge: gate/up on the batch-0 token half while the
        # second reduce-scatter flies ============
        hp_cm = tc.tile_pool(name="ht", bufs=1)
        hp = hp_cm.__enter__()
        gw_cm = tc.tile_pool(name="gw", bufs=2)
        gw = gw_cm.__enter__()
        bp_cm = tc.tile_pool(name="bp", bufs=2, space="PSUM")
        bp = bp_cm.__enter__()
        gs_cm = tc.tile_pool(name="gs", bufs=3)
        gsp = gs_cm.__enter__()
        ht = [hp.tile([P, TOK], BF, tag=f"h{i}", name=f"h{i}") for i in range(FB)]
        xTf = resid['xTf']
        xnT = resid['xnT']
        rsp = resid['rsp']
        def load_gu(qs):
            wgt_a = gw.tile([P, DC * 4 * P], BF, tag="wg", name="wgt_a")
            wut_a = gw.tile([P, DC * 4 * P], BF, tag="wu", name="wut_a")
            nc.sync.dma_start(
                out=wgt_a[:].rearrange("p (c f) -> p c f", c=DC),
                in_=wg[:, qs].rearrange("(c p) f -> p c f", p=P))
            nc.sync.dma_start(
                out=wut_a[:].rearrange("p (c f) -> p c f", c=DC),
                in_=wu[:, qs].rearrange("(c p) f -> p c f", p=P))
            W = 4 * P
            return ([wgt_a[:, i * W:(i + 1) * W] for i in range(DC)],
                    [wut_a[:, i * W:(i + 1) * W] for i in range(DC)])

        for qd in range(NBRQ):
            qs = slice(qd * 4 * P, (qd + 1) * 4 * P)
            wgt, wut = load_gu(qs)
            for j in range(4):
                fb = qd * 4 + j
                g0 = bp.tile([P, TOKB], F32, tag="g", name="g0")
                u0 = bp.tile([P, TOKB], F32, tag="u", name="u0")
                for dc in range(DC):
                    nc.tensor.matmul(g0[:], wgt[dc][:, j * P:(j + 1) * P],
                                     xnT[dc][:, 0:TOKB],
                                     start=(dc == 0), stop=(dc == DC - 1))
                    nc.tensor.matmul(u0[:], wut[dc][:, j * P:(j + 1) * P],
                                     xnT[dc][:, 0:TOKB],
                                     start=(dc == 0), stop=(dc == DC - 1))
                hs0 = gsp.tile([P, TOKB], BF, tag="hs0", name="hs0")
                nc.scalar.activation(hs0[:], g0[:], AF.Silu)
                nc.vector.tensor_mul(ht[fb][:, 0:TOKB], hs0[:], u0[:])

        # ============ residual + rms tail for batch 1 ============
        resid['rpp'] = bp
        resid['rtag'] = "rs"
        emit_resid_half(1)

        # ============ FFN gate/up (rest) ============
        NQD = FB // 4
        for qd in range(NQD):
            qs = slice(qd * 4 * P, (qd + 1) * 4 * P)
            wgt, wut = load_gu(qs)
            if qd < NBRQ:
                # batch-1 half only; batch-0 half came from the bridge
                for j in range(4):
                    fb = qd * 4 + j
                    g_ps = bp.tile([P, TOKB], F32, tag="g", name="g2")
                    u_ps = bp.tile([P, TOKB], F32, tag="u", name="u2")
                    for dc in range(DC):
                        nc.tensor.matmul(g_ps[:], wgt[dc][:, j * P:(j + 1) * P],
                                         xnT[dc][:, TOKB:TOK],
                                         start=(dc == 0), stop=(dc == DC - 1))
                        nc.tensor.matmul(u_ps[:], wut[dc][:, j * P:(j + 1) * P],
                                         xnT[dc][:, TOKB:TOK],
                                         start=(dc == 0), stop=(dc == DC - 1))
                    hs = gsp.tile([P, TOKB], BF, tag="hs2", name="hs")
                    nc.scalar.activation(hs[:], g_ps[:], AF.Silu)
                    nc.vector.tensor_mul(ht[fb][:, TOKB:TOK], hs[:], u_ps[:])
                continue
            for j in range(4):
                fb = qd * 4 + j
                g_ps = bp.tile([P, TOK], F32, tag="g", name="g")
                u_ps = bp.tile([P, TOK], F32, tag="u", name="u")
                for dc in range(DC):
                    nc.tensor.matmul(g_ps[:], wgt[dc][:, j * P:(j + 1) * P],
                                     xnT[dc][:],
                                     start=(dc == 0), stop=(dc == DC - 1))
                    nc.tensor.matmul(u_ps[:], wut[dc][:, j * P:(j + 1) * P],
                                     xnT[dc][:],
                                     start=(dc == 0), stop=(dc == DC - 1))
                hs = gsp.tile([P, TOK], BF, tag="hs", name="hs")
                nc.scalar.activation(hs[:], g_ps[:], AF.Silu)
                nc.vector.tensor_mul(ht[fb][:], hs[:], u_ps[:])
        gs_cm.__exit__(None, None, None)
        bp_cm.__exit__(None, None, None)
        gw_cm.__exit__(None, None, None)

        # ============ FFN down + residual ============
        with tc.tile_pool(name="dw", bufs=3) as dw, \
             tc.tile_pool(name="dp", bufs=2, space="PSUM") as dps_p, \
             tc.tile_pool(name="ds", bufs=2) as dsp:
            FBG = 8      # wd fb-chunks loaded per DMA
            for dbg in range(4):
                dps = [dps_p.tile([P, TOK], F32, tag=f"d{j}", name=f"d{j}")
                       for j in range(4)]
                for fg in range(FB // FBG):
                    wdt_a = dw.tile([P, FBG * 4 * P], BF, tag="wd",
                                    name="wdt_a")
                    nc.sync.dma_start(
                        out=wdt_a[:].rearrange("p (c f) -> p c f", c=FBG),
                        in_=wd[fg * FBG * P:(fg + 1) * FBG * P,
                               dbg * 4 * P:(dbg + 1) * 4 * P]
                            .rearrange("(c p) f -> p c f", p=P))
                    for i in range(FBG):
                        fb = fg * FBG + i
                        for j in range(4):
                            nc.tensor.matmul(
                                dps[j][:],
                                wdt_a[:, i * 4 * P + j * P:
                                      i * 4 * P + (j + 1) * P],
                                ht[fb][:],
                                start=(fb == 0), stop=(fb == FB - 1))
                y_all = dsp.tile([P, 4 * TOK], F32, tag="y", name="y_all")
                for j in range(4):
                    db = dbg * 4 + j
                    nc.vector.tensor_add(y_all[:, j * TOK:(j + 1) * TOK],
                                         dps[j][:], xTf[db][:])
                nc.gpsimd.dma_start(
                    out=yT[dbg * 4 * P:(dbg + 1) * 4 * P, :]
                        .rearrange("(c p) f -> p c f", p=P),
                    in_=y_all[:].rearrange("p (c f) -> p c f", c=4))
        hp_cm.__exit__(None, None, None)
        rs_cm.__exit__(None, None, None)
        xf_cm.__exit__(None, None, None)
        attn_cm.__exit__(None, None, None)

    prune_redundant_waits(nc, verbose=True)
    nc.compile()
    return nc


# --------------------------------------------------------------------------
# Host orchestration
# --------------------------------------------------------------------------
_prog_cache = {}


def _get(key, builder, *args):
    if key not in _prog_cache:
        _prog_cache[key] = builder(*args)
    return _prog_cache[key]


def _bf(x):
    return np.ascontiguousarray(np.asarray(x, dtype=np.float32)).astype(
        ml_dtypes.bfloat16)


def _rope_tables(S, dim):
    freqs = 1.0 / (THETA ** (np.arange(0, dim, 2, dtype=np.float32) / dim))
    f = np.arange(S, dtype=np.float32)[:, None] * freqs[None, :]
    cos = np.repeat(np.cos(f), 2, axis=-1).astype(np.float32)
    sin = np.repeat(np.sin(f), 2, axis=-1).astype(np.float32)
    return cos, sin


def _rot_lhsT(dim):
    rt = np.zeros((dim, dim), np.float32)
    for i in range(dim // 2):
        rt[2 * i + 1, 2 * i] = -1.0
        rt[2 * i, 2 * i + 1] = 1.0
    return rt


def _timed_run(nc, in_maps, reps=200):
    """Execute on all cores with device-resident inputs; time warm reps."""
    import time as _time
    import jax
    from jax.sharding import Mesh, PartitionSpec, NamedSharding
    from jax.experimental.shard_map import shard_map
    from concourse import bass2jax as b2j
    from concourse import mybir as _mb

    b2j.install_neuronx_cc_hook()
    n_cores = len(in_maps)
    in_names, out_names, out_avals, zero_outs = [], [], [], []
    for alloc in nc.m.functions[0].allocations:
        if not isinstance(alloc, _mb.MemoryLocationSet):
            continue
        name = alloc.memorylocations[0].name
        pid_name = nc.partition_id_tensor.name if nc.partition_id_tensor else None
        if alloc.kind == "ExternalInput":
            if name != pid_name:
                in_names.append(name)
        elif alloc.kind == "ExternalOutput":
            out_names.append(name)
            shape = tuple(alloc.tensor_shape)
            dtype = _mb.dt.np(alloc.dtype)
            out_avals.append(jax.core.ShapedArray(shape, dtype))
            zero_outs.append(np.zeros(shape, dtype))
    n_params = len(in_names)
    n_outs = len(out_avals)
    all_names = list(in_names) + list(out_names)
    if nc.partition_id_tensor is not None:
        all_names.append(nc.partition_id_tensor.name)

    def _body(*args):
        operands = list(args)
        if nc.partition_id_tensor is not None:
            operands.append(b2j.partition_id_tensor())
        outs = b2j._bass_exec_p.bind(
            *operands, out_avals=tuple(out_avals), in_names=tuple(all_names),
            out_names=tuple(out_names), lowering_input_output_aliases=(),
            sim_require_finite=True, sim_require_nnan=True, nc=nc)
        return tuple(outs)

    devices = jax.devices()[:n_cores]
    mesh = Mesh(np.asarray(devices), ("core",))
    donate = tuple(range(n_params, n_params + n_outs))
    sharded = jax.jit(
        shard_map(_body, mesh=mesh,
                  in_specs=(PartitionSpec("core"),) * (n_params + n_outs),
                  out_specs=(PartitionSpec("core"),) * n_outs,
                  check_rep=False),
        donate_argnums=donate, keep_unused=True)
    sh = NamedSharding(mesh, PartitionSpec("core"))
    dev_in = [jax.device_put(
        np.concatenate([np.asarray(in_maps[c][n]) for c in range(n_cores)], axis=0), sh)
        for n in in_names]
    dz = [jax.device_put(
        np.zeros((n_cores * z.shape[0], *z.shape[1:]), z.dtype), sh)
        for z in zero_outs]
    jax.block_until_ready(dz)
    outs = sharded(*dev_in, *dz)
    jax.block_until_ready(outs)
    best_total = None
    for _trial in range(5):
        zsets = [[jax.device_put(
            np.zeros((n_cores * z.shape[0], *z.shape[1:]), z.dtype), sh)
            for z in zero_outs] for _ in range(reps)]
        jax.block_until_ready(zsets)
        t0 = _time.perf_counter()
        last = None
        for k in range(reps):
            last = sharded(*dev_in, *zsets[k])
        jax.block_until_ready(last)
        total = _time.perf_counter() - t0
        if best_total is None or total < best_total:
            best_total = total
    best = best_total / reps
    results = [
        {name: np.asarray(outs[i]).reshape(n_cores, *out_avals[i].shape)[c]
         for i, name in enumerate(out_names)}
        for c in range(n_cores)]
    return results, best


_last_exec_ns = []


class _Res:
    def __init__(self, results):
        self.results = results


def _run(nc, in_maps, trace=False):
    if trace:
        results, secs = _timed_run(nc, in_maps)
        _last_exec_ns.append(int(secs * 1e9))
        return _Res(results)
    res = run_bass_kernel_spmd(nc, in_maps, list(range(len(in_maps))))
    _last_exec_ns.append(res.exec_time_ns)
    return res


def kernel(query, key_value, g_q, g_kv, g_ffn, w_qc, w_kc, w_qr, w_kr, w_v,
           w_o, w_gate, w_up, w_down, _trace=True):
    query = np.asarray(query, np.float32)
    key_value = np.asarray(key_value, np.float32)
    Bq, Mq, _ = query.shape
    Nq = key_value.shape[1]
    HPC = N_H // NCORES

    g_q = np.asarray(g_q, np.float32)[:, None]
    g_kv = np.asarray(g_kv, np.float32)[:, None]
    g_ffn = np.asarray(g_ffn, np.float32)[:, None]
    wqc = np.asarray(w_qc, np.float32) * g_q
    wqr_f = np.asarray(w_qr, np.float32) * g_q
    wkc = np.asarray(w_kc, np.float32) * g_kv
    wkr_f = np.asarray(w_kr, np.float32) * g_kv
    wv_f = np.asarray(w_v, np.float32) * g_kv
    wo_f = np.asarray(w_o, np.float32)
    wgate = _bf(np.asarray(w_gate, np.float32) * g_ffn)
    wup = _bf(np.asarray(w_up, np.float32) * g_ffn)
    wdown = _bf(np.asarray(w_down, np.float32))

    qT = _bf(query.transpose(0, 2, 1))
    kvT = _bf(key_value.transpose(0, 2, 1))
    cos, sin = _rope_tables(max(Mq, Nq), D_R)
    cos2T = np.ascontiguousarray(np.vstack([cos[:Mq].T] * HPC)).astype(np.float16)
    sin2T = np.ascontiguousarray(np.vstack([sin[:Mq].T] * HPC)).astype(np.float16)
    rot2T = _bf(np.kron(np.eye(HPC, dtype=np.float32), _rot_lhsT(D_R)))

    del _last_exec_ns[:]
    nc1 = _get(("fusedv2", Bq, Mq, Nq, D, HPC), build_fused, Bq, Mq, Nq, D, HPC)
    EI = TOK // 2    # tokens per (core, batch): an eighth of each batch
    in_maps = []
    for c in range(NCORES):
        hs = slice(c * HPC * D_H, (c + 1) * HPC * D_H)
        rs = slice(c * HPC * D_R, (c + 1) * HPC * D_R)
        sl = slice(c * EI, (c + 1) * EI)
        qres = np.ascontiguousarray(
            np.concatenate([query[0, sl, :].T, query[1, sl, :].T],
                           axis=1)).astype(np.float16)
        in_maps.append({
            "qT": qT, "kvT": kvT,
            "wq": _bf(wqc[:, hs]), "wqr": _bf(wqr_f[:, rs]),
            "wk": _bf(wkc[:, hs]), "wkr": _bf(wkr_f[:, rs]),
            "wv": _bf(wv_f[:, hs]),
            "wo": np.ascontiguousarray(wo_f[hs, :]).astype(np.float16),
            "cos2T": cos2T, "sin2T": sin2T, "rot2T": rot2T,
            "qres": qres, "wg": wgate, "wu": wup, "wd": wdown,
        })
    res = _run(nc1, in_maps, trace=_trace)

    y = np.empty((Bq, Mq, D), np.float32)
    for c in range(NCORES):
        sl = slice(c * EI, (c + 1) * EI)
        yT_c = res.results[c]["yT"]
        y[0, sl, :] = yT_c[:, :EI].T
        y[1, sl, :] = yT_c[:, EI:].T
    return y
